# revision 37
# baseline (speedup 1.0000x reference)
"""Trainium2 Bass kernel for the sparse segment-softmax attention module.

Math: the reference computes, per nnz k,
    out[k] = segment_softmax((q1[b,i] + q2[b,j]) . v)  over segments (b, i).
Within a segment (fixed b, i), the q1[b,i].v term is constant and cancels in
softmax (shift invariance), as does the b2.v bias constant.  Hence
    out[k] = exp(u2[b, j_k]) / sum_{d in seg} exp(u2[b, j_d]),
    u2[b, n] = t2[b, n, :] . g,   g = W2^T v.

Device work per NeuronCore (2 batches of the 16, data-parallel over 8 cores):
  - stream t2 shard (4 MB; batch-0 tiles first, batch-1 backpressured via
    tile-pool reuse so batch 0 gets the full HBM ramp)
  - fused multiply+reduce on DVE+ACT -> u2acc [128, 4]
  - per-column: PE transpose, exp fused into the PSUM->SBUF copy, PE
    ones-outer-product to replicate the row across partitions -> table
  - gather exp(u2)[j] with the hardware pool-engine IndirectCopy in two
    1024-position halves (ISA dst limit); each position serves all 8 Q7
    groups in parallel.  The pool queue serializes a ~28ns/position
    post-drain per IC, so the last IC's drain is overlapped with the
    normalize/store tail.
  - compact the group-redundant gather output with one SBUF->SBUF DMA per
    half, windowed softmax normalize on DVE, store.
"""

import os
from contextlib import ExitStack

import numpy as np

B = 16
N1 = 512
N2 = 512
F2 = 1024
DEG = 32
NNZ = B * N1 * DEG
NCORES = 8
BPC = B // NCORES  # batches per core

_CACHE: dict = {}


def _build_program():
    import concourse.bacc as bacc
    import concourse.mybir as mybir
    import concourse.tile as tile

    fp32 = mybir.dt.float32
    bf16 = mybir.dt.bfloat16
    u16 = mybir.dt.uint16

    nc = bacc.Bacc("TRN2", target_bir_lowering=False, debug=False)

    t2s = nc.dram_tensor("t2s", [BPC, N2, F2], fp32, kind="ExternalInput")
    idxs = nc.dram_tensor("idxs", [BPC, 128, 128], u16, kind="ExternalInput")
    gbc = nc.dram_tensor("gbc", [128, F2], fp32, kind="ExternalInput")
    onesr = nc.dram_tensor("onesr", [1, 128], bf16, kind="ExternalInput")
    ident = nc.dram_tensor("ident", [128, 128], fp32, kind="ExternalInput")
    out = nc.dram_tensor("out", [BPC, 128, 128], fp32, kind="ExternalOutput")

    with tile.TileContext(nc) as tc, ExitStack() as ctx:
        constp = ctx.enter_context(tc.tile_pool(name="const", bufs=1))
        t2p = ctx.enter_context(tc.tile_pool(name="t2p", bufs=4))
        prodp = ctx.enter_context(tc.tile_pool(name="prodp", bufs=3))
        smallp = ctx.enter_context(tc.tile_pool(name="small", bufs=2))
        gathp = ctx.enter_context(tc.tile_pool(name="gath", bufs=2))
        psum_tp = ctx.enter_context(tc.tile_pool(name="pst", bufs=2, space="PSUM"))
        psum_rp = ctx.enter_context(tc.tile_pool(name="psr", bufs=2, space="PSUM"))

        # g broadcast + small constants on the scalar (ACT HWDGE) ring so
        # the sync ring belongs entirely to the t2 stream; g first (it
        # gates the mults)
        g_sb = constp.tile([128, F2], fp32)
        nc.scalar.dma_start(g_sb[:], gbc[:])
        ones_t = constp.tile([1, 128], bf16)
        nc.scalar.dma_start(ones_t[:], onesr[:])
        ident_t = constp.tile([128, 128], fp32)
        nc.scalar.dma_start(ident_t[:], ident[:])
        idx_tiles = []
        for b in range(BPC):
            idx_t = constp.tile([128, 128], u16, tag=f"idx{b}", name=f"idx_t{b}")
            nc.scalar.dma_start(idx_t[:], idxs[b])
            idx_tiles.append(idx_t)

        # t2 stream: bufs=4 on the t2 tag means batch 1's tile t reuses the
        # buffer of batch 0's tile t, so its DMA dispatches only once batch
        # 0's mult consumed it -> batch 0 gets the full HBM ramp.
        t2_tiles = []
        for b in range(BPC):
            for t in range(4):
                t2t = t2p.tile([128, F2], fp32, tag="t2", name=f"t2t_{b}_{t}")
                nc.sync.dma_start(t2t[:], t2s[b, 128 * t : 128 * (t + 1), :])
                t2_tiles.append(t2t)

        tables = []
        for b in range(BPC):
            # ---- u2 = t2[b] @ g, one column per 128-row tile ----
            u2acc = smallp.tile([128, 4], fp32, tag="u2acc")
            psum_row = psum_rp.tile([1, 512], fp32, tag="prow")
            for t in range(4):
                t2t = t2_tiles[4 * b + t]
                prod = prodp.tile([128, F2], fp32, tag="prod")
                nc.vector.tensor_tensor(
                    out=prod[:], in0=t2t[:], in1=g_sb[:], op=mybir.AluOpType.mult
                )
                nc.scalar.activation(
                    prod[:],
                    prod[:],
                    func=mybir.ActivationFunctionType.Copy,
                    accum_out=u2acc[:, t : t + 1],
                )
                # column t -> row slice [1, 128t:128t+128] via PE transpose
                nc.tensor.matmul(
                    psum_row[:, 128 * t : 128 * (t + 1)],
                    u2acc[:, t : t + 1],
                    ident_t[:],
                    is_transpose=True,
                )
            # exp fused into the single PSUM evacuation (cast to bf16: the
            # IndirectCopy drain scales with written bytes, so gathering in
            # bf16 halves the dominant pool-engine cost), then one
            # ones-outer-product replicates the row across all partitions
            row_all = smallp.tile([1, 512], bf16, tag="rowall")
            nc.scalar.activation(
                row_all[:], psum_row[:], func=mybir.ActivationFunctionType.Exp
            )
            psum_tab = psum_tp.tile([128, 512], fp32, tag="ptab")
            nc.tensor.matmul(
                psum_tab[:], ones_t[:], row_all[:], start=True, stop=True
            )
            table_b = gathp.tile([128, 512], bf16, tag=f"table{b}")
            nc.scalar.copy(table_b[:], psum_tab[:])
            tables.append(table_b)

        # ---- gathers: all four ICs back-to-back on the pool queue; each
        # IC's data is ready ~1.7us after dispatch, the ~28us post-drain
        # only blocks the next IC, and the last drain overlaps the tail ----
        gouts = []
        for b in range(BPC):
            gout = gathp.tile([128, 2048], bf16, tag=f"gout{b}")
            nc.gpsimd.indirect_copy(
                gout[:, 0:1024], tables[b][:], idx_tiles[b][:, 0:64], True
            )
            nc.gpsimd.indirect_copy(
                gout[:, 1024:2048], tables[b][:], idx_tiles[b][:, 64:128], True
            )
            gouts.append(gout)

        for b in range(BPC):
            gout = gouts[b]
            # ---- compact: one partition per 16-group holds the real data ----
            Cb = smallp.tile([128, 128], bf16, tag=f"Cb{b}")
            gsel = gout[:].rearrange("(g s) k -> g s k", s=16)[:, 0, :]
            nc.sync.dma_start(Cb[:], gsel)
            C = smallp.tile([128, 128], fp32, tag=f"C{b}")
            nc.scalar.copy(C[:], Cb[:])

            # ---- windowed softmax normalize (4 segments x 32 / partition) --
            C3 = C[:].rearrange("p (s d) -> p s d", d=32)
            S = smallp.tile([128, 4], fp32, tag="S")
            nc.vector.tensor_reduce(
                out=S[:], in_=C3, axis=mybir.AxisListType.X, op=mybir.AluOpType.add
            )
            R = smallp.tile([128, 4], fp32, tag="R")
            nc.vector.reciprocal(R[:], S[:])
            O = smallp.tile([128, 128], fp32, tag="O")
            O3 = O[:].rearrange("p (s d) -> p s d", d=32)
            R3 = R[:].unsqueeze(2).broadcast_to((128, 4, 32))
            nc.vector.tensor_tensor(
                out=O3, in0=C3, in1=R3, op=mybir.AluOpType.mult
            )

            nc.sync.dma_start(out[b], O[:])

    nc.compile()
    return nc


def _prep_core_inputs(t2, idx_j, W2, v):
    import ml_dtypes

    g = (W2.T.astype(np.float64) @ v.astype(np.float64)).astype(np.float32)
    gbc = np.ascontiguousarray(np.broadcast_to(g.reshape(1, F2), (128, F2)))
    onesr = np.ones((1, 128), dtype=ml_dtypes.bfloat16)
    ident = np.eye(128, dtype=np.float32)

    j3 = np.ascontiguousarray(idx_j.reshape(B, N1, DEG).astype(np.uint16))
    in_maps = []
    for c in range(NCORES):
        bb = slice(BPC * c, BPC * (c + 1))
        t2s = np.ascontiguousarray(t2[bb])
        idxs = np.empty((BPC, 128, 128), dtype=np.uint16)
        for lb in range(BPC):
            gb = BPC * c + lb
            for grp in range(8):
                stream = j3[gb, 64 * grp : 64 * (grp + 1), :].reshape(2048)
                idxs[lb, 16 * grp : 16 * (grp + 1), :] = stream.reshape(128, 16).T
        in_maps.append(
            {
                "t2s": t2s,
                "idxs": idxs,
                "gbc": gbc,
                "onesr": onesr,
                "ident": ident,
            }
        )
    return in_maps


def kernel(t1, t2, idx_b, idx_i, idx_j, W1, b1, W2, b2, v):
    from concourse.bass_utils import run_bass_kernel_spmd

    if "nc" not in _CACHE:
        _CACHE["nc"] = _build_program()
    nc = _CACHE["nc"]

    in_maps = _prep_core_inputs(
        np.asarray(t2, dtype=np.float32),
        np.asarray(idx_j),
        np.asarray(W2, dtype=np.float32),
        np.asarray(v, dtype=np.float32),
    )
    trace = bool(int(os.environ.get("KERNEL_TRACE", "0")))
    last_err = None
    for _attempt in range(3):
        try:
            res = run_bass_kernel_spmd(nc, in_maps, list(range(NCORES)), trace=trace)
            break
        except Exception as e:  # transient NRT_EXEC_UNIT_UNRECOVERABLE wedges
            last_err = e
    else:
        raise last_err
    _CACHE["last_results"] = res
    outs = [r["out"].reshape(BPC * N1 * DEG) for r in res.results]
    return np.concatenate(outs).astype(np.float32)
